# revision 18
# baseline (speedup 1.0000x reference)
"""HGT layer (heterogeneous graph transformer) on 8 Trainium2 NeuronCores.

v2: engine-balanced redesign (v1 was DVE-bound at 3.1ms: ~8.5 vector ops
per 128-edge block, each paying ~150cyc fixed overhead).

Strategy (dst-partitioned, per sharding hint):
  - Dst nodes partitioned contiguously across 8 cores. Host groups edges
    by dst tile (128 dsts), pads to uniform per-tile block budgets, and
    pre-gathers per-edge data into three flat streams (cols = edge slots,
    in flat schedule order):
      hsT [128=feat, NBF*128]   source features, transposed, bf16
      at  [128=dlane, NBF*128]  one-hot A^T (dst lane per edge), bf16
      Aa  [128=elane, NBF*128]  one-hot A (per block: A[e, b*128+d]), bf16
  - Device, per 4-block group (512 edges), scores in TRANSPOSED layout
    (features on partitions, edges on free axis) so DVE fixed costs
    amortize 4x:
      kT   = watt.T @ hsT4          (PE, wkv stationary)
      qxT  = Q.T @ at4              (PE, per-tile Q stationary)
      prodT= kT * qxT               (DVE, one op per 4 blocks, fp16 out)
      scores[4s+h, e] += Hmask64_s.T @ prodT   (PE, per-head col sums)
    Per 16 groups (superblock): one ACT exp -> escT fp16.
  - Per block (edge-major message path):
      esc_full = escT_slice.T @ HselI_s   (PE: [e,132] = esc expanded to
                 per-head cols 0:128 + raw esc at cols 128:132)
      v        = hsT_b.T @ wmsg           (PE, into bank cols 132:260;
                 cols 260:264 pre-set to 1.0)
      msg      = v132 * esc_full          (DVE, one op: [e,0:128]=v*esc,
                 [e,128:132]=esc)
      aggT[f,d] += msg[:,0:128].T @ A_b   (PE, transposed scatter-sum)
      zT[h,d]  += msg[:,128:132].T @ A_b  (PE, softmax denominators)
  - Per dst tile: rz = exp(-ln(z+eps)) on ACT (no table switches:
    ln/exp/copy share one ACT table set), rz_expT = Hsel4.T @ rzT (PE
    partition-broadcast), T = aggT*rz_expT (DVE), out += T.T@WaT (PE,
    accumulated over relations), blend with skip (DVE stt), DMA out.
  Weight folds as v1: rel_att/rel_msg into Wk/Wv; pri/sqrt(dk) into
  attention weights; sigmoid(skip) and 0.5 cross-relation mean into Wa.
"""

import math
import os

import numpy as np
import ml_dtypes

BF16 = ml_dtypes.bfloat16
FP16 = np.float16

NPAP, NAUT = 100000, 50000
D, H, DK = 128, 4, 32
NCORES = 8
PPC, APC = NPAP // NCORES, NAUT // NCORES  # 12500, 6250
PT = (PPC + 127) // 128  # 98 paper tiles / core
AT = (APC + 127) // 128  # 49 author tiles / core

G = 4           # blocks per score group
NSLOT = 8       # groups per superblock (scores psum tile rows = 4*NSLOT)
CHUNK = 32      # blocks per DMA chunk

LAST_RESULT = {}


def _pack_dsts(degs, n_per_core, ntiles):
    """Degree-aware dst->tile bin packing (per core, 128 dsts/tile) to
    minimize per-tile edge-block budgets. Uniform budgets across cores
    (max). Returns tile_of, lane_of, [nblk_r]."""
    nr = len(degs)
    n_total = len(degs[0])
    caps = []
    for r in range(nr):
        core_tot = np.array([
            int(degs[r][c * n_per_core : (c + 1) * n_per_core].sum())
            for c in range(NCORES)])
        base = max(1, int(core_tot.max() // (ntiles * 128)))
        K = min(ntiles, max(0, -(-(int(core_tot.max()) - ntiles * base * 128)
                                 // 128)) + max(2, ntiles // 8))
        cap = np.full(ntiles, base * 128, np.int64)
        cap[:K] += 128
        caps.append(cap)
    capsA = np.array(caps, np.float64)
    tile_of = np.empty(n_total, np.int64)
    lane_of = np.empty(n_total, np.int64)
    nblk = np.zeros((nr, ntiles), np.int64)
    for c in range(NCORES):
        sl = slice(c * n_per_core, (c + 1) * n_per_core)
        dd = [d[sl].astype(np.int64) for d in degs]
        tot = sum(dd)
        order = np.argsort(-tot, kind="stable")
        cnt = np.zeros((nr, ntiles), np.int64)
        nt = np.zeros(ntiles, np.int64)
        t_of = np.empty(n_per_core, np.int64)
        for i in order:
            d = np.array([x[i] for x in dd], np.float64)[:, None]
            fill = (cnt + d) / capsA
            worst = fill.max(axis=0)
            worst[nt >= 128] = 2e18
            t = int(np.argmin(np.where(worst <= 1.0, worst, worst + 1e17)))
            t_of[i] = t
            nt[t] += 1
            cnt[:, t] += d[:, 0].astype(np.int64)
        tile_of[sl] = t_of
        lane = np.empty(n_per_core, np.int64)
        for t in range(ntiles):
            idx = np.nonzero(t_of == t)[0]
            lane[idx] = np.arange(len(idx))
        lane_of[sl] = lane
        nblk = np.maximum(nblk, -(-cnt // 128))
    return tile_of, lane_of, [nblk[r] for r in range(nr)]


def _edge_slots(src, dst, tile_of, lane_of, n_per_core, ntiles, nblk,
                zero_row):
    """Per-core edge slot assignment grouped by (packed) dst tile."""
    core = dst // n_per_core
    tl = tile_of[dst]
    lane = lane_of[dst].astype(np.int32)

    NB = int(nblk.sum())
    tile_slot0 = np.concatenate([[0], np.cumsum(nblk)]) * 128

    out = []
    for c in range(NCORES):
        sel = np.nonzero(core == c)[0]
        tl_c = tl[sel]
        order = np.argsort(tl_c, kind="stable")
        sel_o = sel[order]
        tl_s = tl_c[order]
        start_of = np.searchsorted(tl_s, np.arange(ntiles))
        within = np.arange(len(sel_o)) - start_of[tl_s]
        slot = tile_slot0[tl_s] + within

        src_slots = np.full(NB * 128, zero_row, np.int64)
        src_slots[slot] = src[sel_o]
        lane_slots = np.full(NB * 128, 255, np.int32)
        lane_slots[slot] = lane[sel_o]
        out.append((src_slots, lane_slots))
    return NB, out


def _prep_dst_type(h, tile_of, lane_of, n_per_core, ntiles):
    hdT, hrow, poss = [], [], []
    for c in range(NCORES):
        ids = np.arange(n_per_core) + c * n_per_core
        pos = tile_of[ids] * 128 + lane_of[ids]
        pad = np.zeros((ntiles * 128, D), np.float32)
        pad[pos] = h[ids]
        t = pad.reshape(ntiles, 128, D)
        hdT.append(np.ascontiguousarray(t.transpose(0, 2, 1)).astype(BF16))
        hrow.append(np.ascontiguousarray(t))
        poss.append(pos)
    return hdT, hrow, poss


def _fold_weights(Wk, Wv, Wq, Wa, rel_att, rel_msg, rel_pri, skip):
    sqrt_dk = math.sqrt(DK)
    rel_ts = [0, 1, 0]  # src type: cites: paper, writes: author, rev: paper
    watt, wmsg = [], []
    for e in range(3):
        ts = rel_ts[e]
        ratt = rel_att[e] * (rel_pri[e][:, None, None] / sqrt_dk)
        wa = np.einsum("hiI,hij->Ihj", Wk[ts].reshape(H, DK, D), ratt).reshape(D, D)
        wm = np.einsum("hiI,hij->Ihj", Wv[ts].reshape(H, DK, D), rel_msg[e]).reshape(
            D, D
        )
        watt.append(np.ascontiguousarray(wa).astype(BF16))
        wmsg.append(np.ascontiguousarray(wm).astype(BF16))
    wq = [np.ascontiguousarray(Wq[t].T).astype(BF16) for t in range(2)]
    alpha = 1.0 / (1.0 + np.exp(-skip.astype(np.float64)))
    waT = [
        np.ascontiguousarray(Wa[0].T * alpha[0] * 0.5).astype(BF16),
        np.ascontiguousarray(Wa[1].T * alpha[1]).astype(BF16),
    ]
    return watt, wmsg, wq, waT, alpha


def _build_schedule(nblk_c, nblk_w, nblk_r):
    """Flat block schedule. Returns runs list and per-relation block->flat
    column mapping pieces."""
    runs = []  # (rel, ttype, tile, nb, flat_off, rel_off)
    flat = 0
    for t in range(PT):
        for rel, nblk in ((0, nblk_c), (1, nblk_w)):
            nb = int(nblk[t])
            rel_off = int(nblk[:t].sum())
            if nb:
                runs.append((rel, 0, t, nb, flat, rel_off))
                flat += nb
    for t in range(AT):
        nb = int(nblk_r[t])
        rel_off = int(nblk_r[:t].sum())
        if nb:
            runs.append((2, 1, t, nb, flat, rel_off))
            flat += nb
    return runs, flat


def kernel(**inputs):
    from concourse import bacc, bass, mybir, tile
    from concourse.bass_utils import run_bass_kernel_spmd

    inp = {k: np.asarray(v) for k, v in inputs.items()}
    h_paper = inp["h_paper"].astype(np.float32)
    h_author = inp["h_author"].astype(np.float32)
    for bname in ("bk", "bq", "bv", "ba"):
        assert not np.any(inp[bname]), f"nonzero bias {bname} unsupported"

    watt, wmsg, wq, waT, alpha = _fold_weights(
        inp["Wk"].astype(np.float32), inp["Wv"].astype(np.float32),
        inp["Wq"].astype(np.float32), inp["Wa"].astype(np.float32),
        inp["rel_att"].astype(np.float32), inp["rel_msg"].astype(np.float32),
        inp["rel_pri"].astype(np.float32), inp["skip"].astype(np.float32),
    )

    hp_ext = np.concatenate([h_paper, np.zeros((1, D), np.float32)], 0)
    ha_ext = np.concatenate([h_author, np.zeros((1, D), np.float32)], 0)

    deg_c = np.bincount(inp["cites_dst"], minlength=NPAP).astype(np.int64)
    deg_w = np.bincount(inp["writes_dst"], minlength=NPAP).astype(np.int64)
    deg_r = np.bincount(inp["rev_dst"], minlength=NAUT).astype(np.int64)
    tile_p, lane_p, (nblk_c, nblk_w) = _pack_dsts([deg_c, deg_w], PPC, PT)
    tile_a, lane_a, (nblk_r,) = _pack_dsts([deg_r], APC, AT)

    NBC, slots_c = _edge_slots(
        inp["cites_src"].astype(np.int64), inp["cites_dst"].astype(np.int64),
        tile_p, lane_p, PPC, PT, nblk_c, NPAP)
    NBW, slots_w = _edge_slots(
        inp["writes_src"].astype(np.int64), inp["writes_dst"].astype(np.int64),
        tile_p, lane_p, PPC, PT, nblk_w, NAUT)
    NBR, slots_r = _edge_slots(
        inp["rev_src"].astype(np.int64), inp["rev_dst"].astype(np.int64),
        tile_a, lane_a, APC, AT, nblk_r, NPAP)

    runs, NBF = _build_schedule(nblk_c, nblk_w, nblk_r)

    hdT_p, hrow_p, pos_p = _prep_dst_type(h_paper, tile_p, lane_p, PPC, PT)
    hdT_a, hrow_a, pos_a = _prep_dst_type(h_author, tile_a, lane_a, APC, AT)

    # -------- per-core flat streams in schedule order --------
    lane128 = np.arange(128, dtype=np.int32)
    hs_cores, at_cores, Aa_cores = [], [], []
    for c in range(NCORES):
        rel_data = []
        for (h_ext, slots) in ((hp_ext, slots_c), (ha_ext, slots_w),
                               (hp_ext, slots_r)):
            src_slots, lane_slots = slots[c]
            hsT = np.ascontiguousarray(h_ext[src_slots].T).astype(BF16)
            at = (lane128[:, None] == lane_slots[None, :]).astype(BF16)
            nb = len(lane_slots) // 128
            Ab = (lane_slots.reshape(nb, 128)[:, :, None] == lane128).astype(BF16)
            Aa = np.ascontiguousarray(
                Ab.transpose(1, 0, 2).reshape(128, nb * 128))
            rel_data.append((hsT, at, Aa))
        hs_parts, at_parts, Aa_parts = [], [], []
        for (rel, _tt, _t, nb, _f, rel_off) in runs:
            sl = slice(rel_off * 128, (rel_off + nb) * 128)
            hs_parts.append(rel_data[rel][0][:, sl])
            at_parts.append(rel_data[rel][1][:, sl])
            Aa_parts.append(rel_data[rel][2][:, sl])
        hs_cores.append(np.ascontiguousarray(np.concatenate(hs_parts, 1)))
        at_cores.append(np.ascontiguousarray(np.concatenate(at_parts, 1)))
        Aa_cores.append(np.ascontiguousarray(np.concatenate(Aa_parts, 1)))

    # -------- groups (cut at run & chunk boundaries, size <= G) --------
    # block flat idx -> (run idx, j within run)
    groups = []  # (flat_start, n, rel, ttype, tile, run_first, run_last)
    for (rel, tt, t, nb, f0, _ro) in runs:
        i = 0
        while i < nb:
            fs = f0 + i
            n = min(G, nb - i, ((fs // CHUNK) + 1) * CHUNK - fs)
            groups.append(
                (fs, n, rel, tt, t, i == 0, i + n == nb))
            i += n
    NG = len(groups)
    NSB = (NG + NSLOT - 1) // NSLOT

    # -------- build SPMD program --------
    nc = bacc.Bacc("TRN2", target_bir_lowering=False, debug=False,
                   num_devices=NCORES)
    dt = mybir.dt

    d_hs = nc.dram_tensor("hs_flat", [128, NBF * 128], dt.bfloat16,
                          kind="ExternalInput")
    d_at = nc.dram_tensor("at_flat", [128, NBF * 128], dt.bfloat16,
                          kind="ExternalInput")
    d_Aa = nc.dram_tensor("Aa_flat", [128, NBF * 128], dt.bfloat16,
                          kind="ExternalInput")
    d_hdT = {
        0: nc.dram_tensor("hdT_paper", [PT, 128, 128], dt.bfloat16,
                          kind="ExternalInput"),
        1: nc.dram_tensor("hdT_author", [AT, 128, 128], dt.bfloat16,
                          kind="ExternalInput"),
    }
    d_hrow = {
        0: nc.dram_tensor("hrow_paper", [PT, 128, 128], dt.float32,
                          kind="ExternalInput"),
        1: nc.dram_tensor("hrow_author", [AT, 128, 128], dt.float32,
                          kind="ExternalInput"),
    }
    NOUT = (PT + AT) * 128
    d_out = nc.dram_tensor("out", [NOUT, 128], dt.float32, kind="ExternalOutput")

    d_watt = [nc.inline_tensor(watt[e], name=f"watt{e}") for e in range(3)]
    d_wmsg = [nc.inline_tensor(wmsg[e], name=f"wmsg{e}") for e in range(3)]
    d_wq = [nc.inline_tensor(wq[t], name=f"wq{t}") for t in range(2)]
    d_waT = [nc.inline_tensor(waT[t], name=f"waT{t}") for t in range(2)]

    # Hmask_s [128f, 4*NSLOT]: col m==4s+head(f) -> 1
    hmask_np = []
    headof = (np.arange(128) >> 5)
    for s in range(NSLOT):
        m = (np.arange(4 * NSLOT)[None, :] == (4 * s + headof)[:, None])
        hmask_np.append(m.astype(FP16))
    d_hmask = [nc.inline_tensor(hmask_np[s], name=f"hmask{s}")
               for s in range(NSLOT)]
    # HselZ_s [4*NSLOT, 4]: [k, j] = delta(k == 4s+j)  (esc z-extract)
    hselz_np = []
    for s in range(NSLOT):
        m = np.zeros((4 * NSLOT, 4), FP16)
        for j in range(4):
            m[4 * s + j, j] = 1
        hselz_np.append(m)
    d_hselz = [nc.inline_tensor(hselz_np[s], name=f"hselz{s}")
               for s in range(NSLOT)]
    # Hsel4e [5, 128] f32: rows 0-3 delta(h == head(f)), row 4 = eps
    # (z_expT = Hsel4e.T @ [zT; ones] = z[head(f), d] + eps)
    hsel4_np = np.concatenate([
        (np.arange(4)[:, None] == headof[None, :]).astype(np.float32),
        np.full((1, 128), 1e-30, np.float32)], 0)
    d_hsel4 = nc.inline_tensor(hsel4_np, name="hsel4e")

    from contextlib import ExitStack

    with tile.TileContext(nc) as tc, ExitStack() as _es:
        _p = lambda *a, **k: _es.enter_context(tc.tile_pool(*a, **k))
        cpool = _p(name="const", bufs=1)
        hs_pool = _p(name="hs_st", bufs=5)
        at_pool = _p(name="at_st", bufs=4)
        Aa_pool = _p(name="Aa_st", bufs=4)
        esc_pool = _p(name="escT", bufs=3)
        prod_pool = _p(name="prodT", bufs=3)
        msg_pool = _p(name="msg", bufs=3)
        qxs_pool = _p(name="qxTs", bufs=3)
        escE_pool = _p(name="escE", bufs=3)
        q_pool = _p(name="qsb", bufs=12)
        hdt_pool = _p(name="hdt", bufs=4)
        t_pool = _p(name="tiles", bufs=4)
        k_ps = _p(name="kps", bufs=2, space="PSUM")
        q_ps_pool = _p(name="qps", bufs=1, space="PSUM")
        sc_ps = _p(name="scps", bufs=1, space="PSUM")
        bankA_pool = _p(name="bankA", bufs=1, space="PSUM")
        bankAgg_pool = _p(name="bankAgg", bufs=1, space="PSUM")
        bankZ_pool = _p(name="bankZ", bufs=1, space="PSUM")
        bankD_pool = _p(name="bankD", bufs=1, space="PSUM")
        if True:
            # constants
            s_watt, s_wmsg = [], []
            for e in range(3):
                a = cpool.tile([128, 128], dt.bfloat16, name=f"s_watt{e}")
                nc.sync.dma_start(out=a[:], in_=d_watt[e][:])
                s_watt.append(a)
                b = cpool.tile([128, 128], dt.bfloat16, name=f"s_wmsg{e}")
                nc.sync.dma_start(out=b[:], in_=d_wmsg[e][:])
                s_wmsg.append(b)
            s_wq, s_waT = [], []
            for t in range(2):
                a = cpool.tile([128, 128], dt.bfloat16, name=f"s_wq{t}")
                nc.sync.dma_start(out=a[:], in_=d_wq[t][:])
                s_wq.append(a)
                b = cpool.tile([128, 128], dt.bfloat16, name=f"s_waT{t}")
                nc.sync.dma_start(out=b[:], in_=d_waT[t][:])
                s_waT.append(b)
            _hmask_c, _hselz_c = {}, {}

            def s_hmask(s):
                if s not in _hmask_c:
                    a = cpool.tile([128, 4 * NSLOT], dt.float16,
                                   name=f"s_hmask{s}")
                    nc.sync.dma_start(out=a[:], in_=d_hmask[s][:])
                    _hmask_c[s] = a
                return _hmask_c[s]

            def s_hselz(s):
                if s not in _hselz_c:
                    b = cpool.tile([4 * NSLOT, 4], dt.float16,
                                   name=f"s_hselz{s}")
                    nc.sync.dma_start(out=b[:], in_=d_hselz[s][:])
                    _hselz_c[s] = b
                return _hselz_c[s]

            s_hsel4 = cpool.tile([5, 128], dt.float32, name="s_hsel4")
            nc.sync.dma_start(out=s_hsel4[:], in_=d_hsel4[:])

            # fixed PSUM tiles. PSUM note: matmul start=True clears
            # has_written for the WHOLE bank, so every multi-matmul
            # accumulation (scores, aggT, zT, out_ps pair) must never have
            # another start=True matmul land in its bank mid-accumulation.
            scores = sc_ps.tile([4 * NSLOT, 512], dt.float32,
                                name="scores")
            bankA = bankA_pool.tile([128, 512], dt.float32, name="bankA")
            bankAgg = bankAgg_pool.tile([128, 512], dt.float32, name="bankAgg")
            bankZ = bankZ_pool.tile([128, 512], dt.float32, name="bankZ")
            bankD = bankD_pool.tile([128, 512], dt.float32, name="bankD")
            # bankA layout: v4 [0:512] (one 128-col region per block)
            nc.vector.memset(scores[:, :], 0.0)
            nc.vector.memset(bankZ[0:8, 0:512], 1.0)

            # stream chunk management
            chunk_tiles = {}

            def get_chunk(which, pool, ci):
                key = (which, ci)
                if key in chunk_tiles:
                    return chunk_tiles[key]
                c0 = ci * CHUNK * 128
                w = min(CHUNK * 128, NBF * 128 - c0)
                tl = pool.tile([128, CHUNK * 128], dt.bfloat16, name=which,
                               tag=which)
                src = {"hs": d_hs, "at": d_at, "Aa": d_Aa}[which]
                nc.sync.dma_start(out=tl[:, :w], in_=src[:, c0 : c0 + w])
                chunk_tiles[key] = tl
                return tl

            def chunk_slice(which, pool, fs, n):
                ci, off = divmod(fs, CHUNK)
                tl = get_chunk(which, pool, ci)
                return tl[:, off * 128 : (off + n) * 128]

            # per-tile state
            q_tiles = {}       # (tt, tile) -> Q sbuf tile
            tile_state = {}    # (tt, tile) -> dict(bankC, rels list)

            def emit_q(tt, t):
                key = (tt, t)
                if key in q_tiles:
                    return q_tiles[key]
                hdt = hdt_pool.tile([128, 128], dt.bfloat16, name="hdt",
                                    tag="hdt")
                nc.sync.dma_start(out=hdt[:], in_=d_hdT[tt][t, :, :])
                nc.tensor.matmul(bankD[:, 0:128], lhsT=hdt[:], rhs=s_wq[tt][:],
                                 start=True, stop=True)
                Q = q_pool.tile([128, 128], dt.bfloat16, name="Q", tag="Q")
                nc.scalar.copy(out=Q[:], in_=bankD[:, 0:128])
                q_tiles[key] = Q
                return Q

            tile_seq = [0]

            def get_tile_state(tt, t):
                key = (tt, t)
                if key not in tile_state:
                    tile_state[key] = {"rels": [], "half": 256 * (tile_seq[0] & 1)}
                    tile_seq[0] += 1
                return tile_state[key]

            def finalize_tile(tt, t):
                st = tile_state[(tt, t)]
                rels = st["rels"]
                orow = t * 128 if tt == 0 else (PT + t) * 128
                hrow = t_pool.tile([128, 128], dt.float32, name="hrow",
                                   tag="hrow")
                nc.sync.dma_start(out=hrow[:], in_=d_hrow[tt][t, :, :])
                out_s = t_pool.tile([128, 128], dt.float32, name="out_s",
                                    tag="out_s")
                if rels:
                    nr = len(rels)
                    hf = st["half"]
                    riof = [0, 1, 0]  # bank region per relation
                    c0 = hf + 128 * riof[rels[0]]
                    # zT rows 0-3 + the persistent ones row 4 -> SBUF
                    zT_sb = t_pool.tile([5, 256], dt.float32, name="zT_sb",
                                        tag="zT_sb")
                    nc.scalar.copy(out=zT_sb[:, 0 : 128 * nr],
                                   in_=bankZ[0:5, c0 : c0 + 128 * nr])
                    T_sbs = []
                    for pi, rel in enumerate(rels):
                        ri = riof[rel]
                        # z_expT + eps [f, d] into bankD[256:384] (f32 matmul)
                        nc.tensor.matmul(
                            bankD[:, 256:384], lhsT=s_hsel4[:],
                            rhs=zT_sb[0:5, 128 * pi : 128 * pi + 128],
                            start=True, stop=True)
                        rz_sb = t_pool.tile([128, 128], dt.float32,
                                            name="rz_sb", tag="rz_sb")
                        nc.vector.reciprocal_approx_fast(
                            out=rz_sb[:], in_=bankD[:, 256:384])
                        T_sb = t_pool.tile([128, 128], dt.bfloat16, name="T_sb",
                                           tag="T_sb")
                        nc.vector.tensor_tensor(
                            out=T_sb[:],
                            in0=bankAgg[:, hf + 128 * ri : hf + 128 * ri + 128],
                            in1=rz_sb[:], op=mybir.AluOpType.mult)
                        T_sbs.append(T_sb)
                    # out-MM accumulation pair kept adjacent: no other
                    # start=True matmul may land in bankD between them
                    for pi, T_sb in enumerate(T_sbs):
                        nc.tensor.matmul(bankD[:, 128:256], lhsT=T_sb[:],
                                         rhs=s_waT[tt][:],
                                         start=(pi == 0), stop=(pi == nr - 1))
                    nc.vector.scalar_tensor_tensor(
                        out=out_s[:], in0=hrow[:],
                        scalar=float(1.0 - alpha[tt]), in1=bankD[:, 128:256],
                        op0=mybir.AluOpType.mult, op1=mybir.AluOpType.add)
                else:
                    nc.vector.tensor_scalar(
                        out=out_s[:], in0=hrow[:],
                        scalar1=float(1.0 - alpha[tt]), scalar2=None,
                        op0=mybir.AluOpType.mult)
                nc.sync.dma_start(out=d_out[orow : orow + 128, :], in_=out_s[:])
                del tile_state[(tt, t)]

            # main superblock loop
            for sb in range(NSB):
                g0 = sb * NSLOT
                sb_groups = groups[g0 : g0 + NSLOT]
                ns = len(sb_groups)
                # ---- phase A ----
                for s, (fs, n, rel, tt, t, rfirst, rlast) in enumerate(sb_groups):
                    Q = emit_q(tt, t)
                    ec = n * 128
                    hs4 = chunk_slice("hs", hs_pool, fs, n)
                    at4 = chunk_slice("at", at_pool, fs, n)
                    kT = k_ps.tile([128, 512], dt.float32, name="kT", tag="kT")
                    nc.tensor.matmul(kT[:, :ec], lhsT=s_watt[rel][:], rhs=hs4,
                                     start=True, stop=True)
                    qxT = q_ps_pool.tile([128, 512], dt.float32, name="qxT",
                                         tag="qxT")
                    nc.tensor.matmul(qxT[:, :ec], lhsT=Q[:], rhs=at4,
                                     start=True, stop=True)
                    qxTs = qxs_pool.tile([128, 512], dt.float16,
                                         name="qxTs", tag="qxTs")
                    nc.scalar.copy(out=qxTs[:, :ec], in_=qxT[:, :ec])
                    prodT = prod_pool.tile([128, 512], dt.float16, name="prodT",
                                           tag="prodT")
                    nc.vector.tensor_tensor(out=prodT[:, :ec], in0=kT[:, :ec],
                                            in1=qxTs[:, :ec],
                                            op=mybir.AluOpType.mult)
                    nc.tensor.matmul(scores[:, :ec], lhsT=s_hmask(s)[:],
                                     rhs=prodT[:, :ec],
                                     start=(s == 0), stop=(s == ns - 1))
                # ---- exp (always full 64 rows: unused rows hold finite
                # stale scores; keeps escT NaN-free for the K=64 lhsT) ----
                escT = esc_pool.tile([4 * NSLOT, 512], dt.float16,
                                     name="escT", tag="escT")
                nc.scalar.activation(out=escT[:, :], in_=scores[:, :],
                                     func=mybir.ActivationFunctionType.Exp)
                # ---- phase B ----
                for s, (fs, n, rel, tt, t, rfirst, rlast) in enumerate(sb_groups):
                    st = get_tile_state(tt, t)
                    ri = 0 if rel in (0, 2) else 1
                    if rel not in st["rels"]:
                        st["rels"].append(rel)
                    ec = n * 128
                    # escE [e, 4n] edge-major esc via per-block extract MMs
                    for j in range(n):
                        nc.tensor.matmul(
                            bankD[:, 384 + 4 * j : 388 + 4 * j],
                            lhsT=escT[:, 128 * j : 128 * j + 128],
                            rhs=s_hselz(s)[:], start=True, stop=True)
                        hsb = chunk_slice("hs", hs_pool, fs + j, 1)
                        nc.tensor.matmul(
                            bankA[:, 128 * j : 128 * j + 128], lhsT=hsb,
                            rhs=s_wmsg[rel][:], start=True, stop=True)
                    escE = escE_pool.tile([128, 16], dt.bfloat16, name="escE",
                                          tag="escE")
                    nc.vector.tensor_copy(out=escE[:, 0 : 4 * n],
                                          in_=bankD[:, 384 : 384 + 4 * n])
                    msg4 = msg_pool.tile([128, 512], dt.bfloat16, name="msg4",
                                         tag="msg4")
                    nc.vector.tensor_tensor(
                        out=msg4[:, :ec].rearrange("p (x y) -> p x y", y=32),
                        in0=bankA[:, :ec].rearrange("p (x y) -> p x y", y=32),
                        in1=escE[:, 0 : 4 * n].to_broadcast([128, 4 * n, 32]),
                        op=mybir.AluOpType.mult)
                    for j in range(n):
                        first = rfirst and j == 0
                        last = rlast and j == n - 1
                        Ab = chunk_slice("Aa", Aa_pool, fs + j, 1)
                        hf = st["half"]
                        nc.tensor.matmul(
                            bankAgg[:, hf + 128 * ri : hf + 128 * ri + 128],
                            lhsT=msg4[:, 128 * j : 128 * j + 128], rhs=Ab,
                            start=first, stop=last)
                        nc.tensor.matmul(
                            bankZ[0:4, hf + 128 * ri : hf + 128 * ri + 128],
                            lhsT=escE[:, 4 * j : 4 * j + 4], rhs=Ab,
                            start=first, stop=last)
                    if rlast:
                        # finalize when this was the tile's last relation run
                        is_tile_last = (rel == 2) or (tt == 0 and (
                            rel == 1 or (rel == 0 and nblk_w[t] == 0)))
                        if is_tile_last:
                            finalize_tile(tt, t)

            # tiles with no edges at all: pure skip-blend output
            seen = {(tt, t) for (_r, tt, t, _nb, _f, _ro) in runs}
            for tt, nt in ((0, PT), (1, AT)):
                for t in range(nt):
                    if (tt, t) not in seen:
                        get_tile_state(tt, t)
                        finalize_tile(tt, t)

    nc.compile()

    if os.environ.get("HGT_BUILD_ONLY"):
        return np.zeros((NPAP + NAUT, D), np.float32)

    in_maps = []
    for c in range(NCORES):
        in_maps.append({
            "hs_flat": hs_cores[c], "at_flat": at_cores[c],
            "Aa_flat": Aa_cores[c],
            "hdT_paper": hdT_p[c], "hdT_author": hdT_a[c],
            "hrow_paper": hrow_p[c], "hrow_author": hrow_a[c],
        })

    trace = bool(int(os.environ.get("HGT_TRACE", "0")))
    res = run_bass_kernel_spmd(nc, in_maps, list(range(NCORES)), trace=trace)
    LAST_RESULT["exec_time_ns"] = res.exec_time_ns
    LAST_RESULT["res"] = res
    LAST_RESULT["nc"] = nc
    LAST_RESULT["in_maps"] = in_maps

    out = np.empty((NPAP + NAUT, D), np.float32)
    for c in range(NCORES):
        o = np.asarray(res.results[c]["out"], np.float32)
        out[c * PPC : (c + 1) * PPC] = o[pos_p[c]]
        out[NPAP + c * APC : NPAP + (c + 1) * APC] = o[PT * 128 + pos_a[c]]
    return out


# revision 22
# speedup vs baseline: 1.1530x; 1.1530x over previous
"""HGT layer (heterogeneous graph transformer) on 8 Trainium2 NeuronCores.

v2: engine-balanced redesign (v1 was DVE-bound at 3.1ms: ~8.5 vector ops
per 128-edge block, each paying ~150cyc fixed overhead).

Strategy (dst-partitioned, per sharding hint):
  - Dst nodes partitioned contiguously across 8 cores. Host groups edges
    by dst tile (128 dsts), pads to uniform per-tile block budgets, and
    pre-gathers per-edge data into three flat streams (cols = edge slots,
    in flat schedule order):
      hsT [128=feat, NBF*128]   source features, transposed, bf16
      at  [128=dlane, NBF*128]  one-hot A^T (dst lane per edge), bf16
      Aa  [128=elane, NBF*128]  one-hot A (per block: A[e, b*128+d]), bf16
  - Device, per 4-block group (512 edges), scores in TRANSPOSED layout
    (features on partitions, edges on free axis) so DVE fixed costs
    amortize 4x:
      kT   = watt.T @ hsT4          (PE, wkv stationary)
      qxT  = Q.T @ at4              (PE, per-tile Q stationary)
      prodT= kT * qxT               (DVE, one op per 4 blocks, fp16 out)
      scores[4s+h, e] += Hmask64_s.T @ prodT   (PE, per-head col sums)
    Per 16 groups (superblock): one ACT exp -> escT fp16.
  - Per block (edge-major message path):
      esc_full = escT_slice.T @ HselI_s   (PE: [e,132] = esc expanded to
                 per-head cols 0:128 + raw esc at cols 128:132)
      v        = hsT_b.T @ wmsg           (PE, into bank cols 132:260;
                 cols 260:264 pre-set to 1.0)
      msg      = v132 * esc_full          (DVE, one op: [e,0:128]=v*esc,
                 [e,128:132]=esc)
      aggT[f,d] += msg[:,0:128].T @ A_b   (PE, transposed scatter-sum)
      zT[h,d]  += msg[:,128:132].T @ A_b  (PE, softmax denominators)
  - Per dst tile: rz = exp(-ln(z+eps)) on ACT (no table switches:
    ln/exp/copy share one ACT table set), rz_expT = Hsel4.T @ rzT (PE
    partition-broadcast), T = aggT*rz_expT (DVE), out += T.T@WaT (PE,
    accumulated over relations), blend with skip (DVE stt), DMA out.
  Weight folds as v1: rel_att/rel_msg into Wk/Wv; pri/sqrt(dk) into
  attention weights; sigmoid(skip) and 0.5 cross-relation mean into Wa.
"""

import math
import os

import numpy as np
import ml_dtypes

BF16 = ml_dtypes.bfloat16
FP16 = np.float16

NPAP, NAUT = 100000, 50000
D, H, DK = 128, 4, 32
NCORES = 8
PPC, APC = NPAP // NCORES, NAUT // NCORES  # 12500, 6250
PT = (PPC + 127) // 128  # 98 paper tiles / core
AT = (APC + 127) // 128  # 49 author tiles / core

G = 4           # blocks per score group
NSLOT = 16      # groups per superblock (scores psum tile rows = 4*NSLOT)
CHUNK = 32      # blocks per DMA chunk

LAST_RESULT = {}


def _pack_dsts(degs, n_per_core, ntiles):
    """Degree-aware dst->tile bin packing (per core, 128 dsts/tile) to
    minimize per-tile edge-block budgets. Uniform budgets across cores
    (max). Returns tile_of, lane_of, [nblk_r]."""
    nr = len(degs)
    n_total = len(degs[0])
    caps = []
    for r in range(nr):
        core_tot = np.array([
            int(degs[r][c * n_per_core : (c + 1) * n_per_core].sum())
            for c in range(NCORES)])
        base = max(1, int(core_tot.max() // (ntiles * 128)))
        K = min(ntiles, max(0, -(-(int(core_tot.max()) - ntiles * base * 128)
                                 // 128)) + max(2, ntiles // 8))
        cap = np.full(ntiles, base * 128, np.int64)
        cap[:K] += 128
        caps.append(cap)
    capsA = np.array(caps, np.float64)
    tile_of = np.empty(n_total, np.int64)
    lane_of = np.empty(n_total, np.int64)
    nblk = np.zeros((nr, ntiles), np.int64)
    for c in range(NCORES):
        sl = slice(c * n_per_core, (c + 1) * n_per_core)
        dd = [d[sl].astype(np.int64) for d in degs]
        tot = sum(dd)
        order = np.argsort(-tot, kind="stable")
        cnt = np.zeros((nr, ntiles), np.int64)
        nt = np.zeros(ntiles, np.int64)
        t_of = np.empty(n_per_core, np.int64)
        for i in order:
            d = np.array([x[i] for x in dd], np.float64)[:, None]
            fill = (cnt + d) / capsA
            worst = fill.max(axis=0)
            worst[nt >= 128] = 2e18
            t = int(np.argmin(np.where(worst <= 1.0, worst, worst + 1e17)))
            t_of[i] = t
            nt[t] += 1
            cnt[:, t] += d[:, 0].astype(np.int64)
        tile_of[sl] = t_of
        lane = np.empty(n_per_core, np.int64)
        for t in range(ntiles):
            idx = np.nonzero(t_of == t)[0]
            lane[idx] = np.arange(len(idx))
        lane_of[sl] = lane
        nblk = np.maximum(nblk, -(-cnt // 128))
    return tile_of, lane_of, [nblk[r] for r in range(nr)]


def _edge_slots(src, dst, tile_of, lane_of, n_per_core, ntiles, nblk,
                zero_row):
    """Per-core edge slot assignment grouped by (packed) dst tile."""
    core = dst // n_per_core
    tl = tile_of[dst]
    lane = lane_of[dst].astype(np.int32)

    NB = int(nblk.sum())
    tile_slot0 = np.concatenate([[0], np.cumsum(nblk)]) * 128

    out = []
    for c in range(NCORES):
        sel = np.nonzero(core == c)[0]
        tl_c = tl[sel]
        order = np.argsort(tl_c, kind="stable")
        sel_o = sel[order]
        tl_s = tl_c[order]
        start_of = np.searchsorted(tl_s, np.arange(ntiles))
        within = np.arange(len(sel_o)) - start_of[tl_s]
        slot = tile_slot0[tl_s] + within

        src_slots = np.full(NB * 128, zero_row, np.int64)
        src_slots[slot] = src[sel_o]
        lane_slots = np.full(NB * 128, 255, np.int32)
        lane_slots[slot] = lane[sel_o]
        out.append((src_slots, lane_slots))
    return NB, out


def _prep_dst_type(h, tile_of, lane_of, n_per_core, ntiles):
    hdT, hrow, poss = [], [], []
    for c in range(NCORES):
        ids = np.arange(n_per_core) + c * n_per_core
        pos = tile_of[ids] * 128 + lane_of[ids]
        pad = np.zeros((ntiles * 128, D), np.float32)
        pad[pos] = h[ids]
        t = pad.reshape(ntiles, 128, D)
        hdT.append(np.ascontiguousarray(t.transpose(0, 2, 1)).astype(BF16))
        hrow.append(np.ascontiguousarray(t))
        poss.append(pos)
    return hdT, hrow, poss


def _fold_weights(Wk, Wv, Wq, Wa, rel_att, rel_msg, rel_pri, skip):
    sqrt_dk = math.sqrt(DK)
    rel_ts = [0, 1, 0]  # src type: cites: paper, writes: author, rev: paper
    watt, wmsg = [], []
    for e in range(3):
        ts = rel_ts[e]
        ratt = rel_att[e] * (rel_pri[e][:, None, None] / sqrt_dk)
        wa = np.einsum("hiI,hij->Ihj", Wk[ts].reshape(H, DK, D), ratt).reshape(D, D)
        wm = np.einsum("hiI,hij->Ihj", Wv[ts].reshape(H, DK, D), rel_msg[e]).reshape(
            D, D
        )
        watt.append(np.ascontiguousarray(wa).astype(BF16))
        wmsg.append(np.ascontiguousarray(wm).astype(BF16))
    wq = [np.ascontiguousarray(Wq[t].T).astype(BF16) for t in range(2)]
    alpha = 1.0 / (1.0 + np.exp(-skip.astype(np.float64)))
    waT = [
        np.ascontiguousarray(Wa[0].T * alpha[0] * 0.5).astype(BF16),
        np.ascontiguousarray(Wa[1].T * alpha[1]).astype(BF16),
    ]
    return watt, wmsg, wq, waT, alpha


def _build_schedule(nblk_c, nblk_w, nblk_r):
    """Flat block schedule. Returns runs list and per-relation block->flat
    column mapping pieces."""
    runs = []  # (rel, ttype, tile, nb, flat_off, rel_off)
    flat = 0
    for t in range(PT):
        for rel, nblk in ((0, nblk_c), (1, nblk_w)):
            nb = int(nblk[t])
            rel_off = int(nblk[:t].sum())
            if nb:
                runs.append((rel, 0, t, nb, flat, rel_off))
                flat += nb
    for t in range(AT):
        nb = int(nblk_r[t])
        rel_off = int(nblk_r[:t].sum())
        if nb:
            runs.append((2, 1, t, nb, flat, rel_off))
            flat += nb
    return runs, flat


def kernel(**inputs):
    from concourse import bacc, bass, mybir, tile
    from concourse.bass_utils import run_bass_kernel_spmd

    inp = {k: np.asarray(v) for k, v in inputs.items()}
    h_paper = inp["h_paper"].astype(np.float32)
    h_author = inp["h_author"].astype(np.float32)
    for bname in ("bk", "bq", "bv", "ba"):
        assert not np.any(inp[bname]), f"nonzero bias {bname} unsupported"

    watt, wmsg, wq, waT, alpha = _fold_weights(
        inp["Wk"].astype(np.float32), inp["Wv"].astype(np.float32),
        inp["Wq"].astype(np.float32), inp["Wa"].astype(np.float32),
        inp["rel_att"].astype(np.float32), inp["rel_msg"].astype(np.float32),
        inp["rel_pri"].astype(np.float32), inp["skip"].astype(np.float32),
    )

    hp_ext = np.concatenate([h_paper, np.zeros((1, D), np.float32)], 0)
    ha_ext = np.concatenate([h_author, np.zeros((1, D), np.float32)], 0)

    deg_c = np.bincount(inp["cites_dst"], minlength=NPAP).astype(np.int64)
    deg_w = np.bincount(inp["writes_dst"], minlength=NPAP).astype(np.int64)
    deg_r = np.bincount(inp["rev_dst"], minlength=NAUT).astype(np.int64)
    tile_p, lane_p, (nblk_c, nblk_w) = _pack_dsts([deg_c, deg_w], PPC, PT)
    tile_a, lane_a, (nblk_r,) = _pack_dsts([deg_r], APC, AT)

    NBC, slots_c = _edge_slots(
        inp["cites_src"].astype(np.int64), inp["cites_dst"].astype(np.int64),
        tile_p, lane_p, PPC, PT, nblk_c, NPAP)
    NBW, slots_w = _edge_slots(
        inp["writes_src"].astype(np.int64), inp["writes_dst"].astype(np.int64),
        tile_p, lane_p, PPC, PT, nblk_w, NAUT)
    NBR, slots_r = _edge_slots(
        inp["rev_src"].astype(np.int64), inp["rev_dst"].astype(np.int64),
        tile_a, lane_a, APC, AT, nblk_r, NPAP)

    runs, NBF = _build_schedule(nblk_c, nblk_w, nblk_r)

    hdT_p, hrow_p, pos_p = _prep_dst_type(h_paper, tile_p, lane_p, PPC, PT)
    hdT_a, hrow_a, pos_a = _prep_dst_type(h_author, tile_a, lane_a, APC, AT)

    # -------- per-core flat streams in schedule order --------
    lane128 = np.arange(128, dtype=np.int32)
    hs_cores, at_cores, Aa_cores = [], [], []
    for c in range(NCORES):
        rel_data = []
        for (h_ext, slots) in ((hp_ext, slots_c), (ha_ext, slots_w),
                               (hp_ext, slots_r)):
            src_slots, lane_slots = slots[c]
            hsT = np.ascontiguousarray(h_ext[src_slots].T).astype(BF16)
            at = (lane128[:, None] == lane_slots[None, :]).astype(BF16)
            nb = len(lane_slots) // 128
            Ab = (lane_slots.reshape(nb, 128)[:, :, None] == lane128).astype(FP16)
            Aa = np.ascontiguousarray(
                Ab.transpose(1, 0, 2).reshape(128, nb * 128))
            rel_data.append((hsT, at, Aa))
        hs_parts, at_parts, Aa_parts = [], [], []
        for (rel, _tt, _t, nb, _f, rel_off) in runs:
            sl = slice(rel_off * 128, (rel_off + nb) * 128)
            hs_parts.append(rel_data[rel][0][:, sl])
            at_parts.append(rel_data[rel][1][:, sl])
            Aa_parts.append(rel_data[rel][2][:, sl])
        hs_cores.append(np.ascontiguousarray(np.concatenate(hs_parts, 1)))
        at_cores.append(np.ascontiguousarray(np.concatenate(at_parts, 1)))
        Aa_cores.append(np.ascontiguousarray(np.concatenate(Aa_parts, 1)))

    # -------- groups (cut at run & chunk boundaries, size <= G) --------
    # block flat idx -> (run idx, j within run)
    groups = []  # (flat_start, n, rel, ttype, tile, run_first, run_last)
    for (rel, tt, t, nb, f0, _ro) in runs:
        i = 0
        while i < nb:
            fs = f0 + i
            n = min(G, nb - i, ((fs // CHUNK) + 1) * CHUNK - fs)
            groups.append(
                (fs, n, rel, tt, t, i == 0, i + n == nb))
            i += n
    NG = len(groups)
    NSB = (NG + NSLOT - 1) // NSLOT

    # -------- build SPMD program --------
    nc = bacc.Bacc("TRN2", target_bir_lowering=False, debug=False,
                   num_devices=NCORES)
    dt = mybir.dt

    d_hs = nc.dram_tensor("hs_flat", [128, NBF * 128], dt.bfloat16,
                          kind="ExternalInput")
    d_at = nc.dram_tensor("at_flat", [128, NBF * 128], dt.bfloat16,
                          kind="ExternalInput")
    d_Aa = nc.dram_tensor("Aa_flat", [128, NBF * 128], dt.float16,
                          kind="ExternalInput")
    d_hdT = {
        0: nc.dram_tensor("hdT_paper", [PT, 128, 128], dt.bfloat16,
                          kind="ExternalInput"),
        1: nc.dram_tensor("hdT_author", [AT, 128, 128], dt.bfloat16,
                          kind="ExternalInput"),
    }
    d_hrow = {
        0: nc.dram_tensor("hrow_paper", [PT, 128, 128], dt.float32,
                          kind="ExternalInput"),
        1: nc.dram_tensor("hrow_author", [AT, 128, 128], dt.float32,
                          kind="ExternalInput"),
    }
    NOUT = (PT + AT) * 128
    d_out = nc.dram_tensor("out", [NOUT, 128], dt.float32, kind="ExternalOutput")

    d_watt = [nc.inline_tensor(watt[e], name=f"watt{e}") for e in range(3)]
    d_wmsg = [nc.inline_tensor(wmsg[e], name=f"wmsg{e}") for e in range(3)]
    d_wq = [nc.inline_tensor(wq[t], name=f"wq{t}") for t in range(2)]
    d_waT = [nc.inline_tensor(waT[t], name=f"waT{t}") for t in range(2)]

    # Hmask_s [128f, 4*NSLOT]: col m==4s+head(f) -> 1
    hmask_np = []
    headof = (np.arange(128) >> 5)
    for s in range(NSLOT):
        m = (np.arange(4 * NSLOT)[None, :] == (4 * s + headof)[:, None])
        hmask_np.append(m.astype(FP16))
    d_hmask = [nc.inline_tensor(hmask_np[s], name=f"hmask{s}")
               for s in range(NSLOT)]
    # Hsel4e [5, 128] f32: rows 0-3 delta(h == head(f)), row 4 = eps
    # (z_expT = Hsel4e.T @ [zT; ones] = z[head(f), d] + eps)
    hsel4_np = np.concatenate([
        (np.arange(4)[:, None] == headof[None, :]).astype(np.float32),
        np.full((1, 128), 1e-30, np.float32)], 0)
    d_hsel4 = nc.inline_tensor(hsel4_np, name="hsel4e")

    from contextlib import ExitStack

    with tile.TileContext(nc) as tc, ExitStack() as _es:
        _p = lambda *a, **k: _es.enter_context(tc.tile_pool(*a, **k))
        cpool = _p(name="const", bufs=1)
        hs_pool = _p(name="hs_st", bufs=5)
        at_pool = _p(name="at_st", bufs=4)
        Aa_pool = _p(name="Aa_st", bufs=4)
        esc_pool = _p(name="escT", bufs=3)
        prod_pool = _p(name="prodT", bufs=3)
        msg_pool = _p(name="msg", bufs=3)
        qxs_pool = _p(name="qxTs", bufs=3)
        escET_pool = _p(name="escET", bufs=3)
        q_pool = _p(name="qsb", bufs=12)
        hdt_pool = _p(name="hdt", bufs=4)
        t_pool = _p(name="tiles", bufs=4)
        k_ps = _p(name="kps", bufs=2, space="PSUM")
        q_ps_pool = _p(name="qps", bufs=1, space="PSUM")
        sc_ps = _p(name="scps", bufs=1, space="PSUM")
        bankA_pool = _p(name="bankA", bufs=1, space="PSUM")
        bankAgg_pool = _p(name="bankAgg", bufs=1, space="PSUM")
        bankZ_pool = _p(name="bankZ", bufs=1, space="PSUM")
        bankD_pool = _p(name="bankD", bufs=1, space="PSUM")
        if True:
            # constants
            s_watt, s_wmsg = [], []
            for e in range(3):
                a = cpool.tile([128, 128], dt.bfloat16, name=f"s_watt{e}")
                nc.sync.dma_start(out=a[:], in_=d_watt[e][:])
                s_watt.append(a)
                b = cpool.tile([128, 128], dt.bfloat16, name=f"s_wmsg{e}")
                nc.sync.dma_start(out=b[:], in_=d_wmsg[e][:])
                s_wmsg.append(b)
            s_wq, s_waT = [], []
            for t in range(2):
                a = cpool.tile([128, 128], dt.bfloat16, name=f"s_wq{t}")
                nc.sync.dma_start(out=a[:], in_=d_wq[t][:])
                s_wq.append(a)
                b = cpool.tile([128, 128], dt.bfloat16, name=f"s_waT{t}")
                nc.sync.dma_start(out=b[:], in_=d_waT[t][:])
                s_waT.append(b)
            _hmask_c = {}

            def s_hmask(s):
                if s not in _hmask_c:
                    a = cpool.tile([128, 4 * NSLOT], dt.float16,
                                   name=f"s_hmask{s}")
                    nc.sync.dma_start(out=a[:], in_=d_hmask[s][:])
                    _hmask_c[s] = a
                return _hmask_c[s]

            s_hsel4 = cpool.tile([5, 128], dt.float32, name="s_hsel4")
            nc.sync.dma_start(out=s_hsel4[:], in_=d_hsel4[:])

            # fixed PSUM tiles. PSUM note: matmul start=True clears
            # has_written for the WHOLE bank, so every multi-matmul
            # accumulation (scores, aggT, zT, out_ps pair) must never have
            # another start=True matmul land in its bank mid-accumulation.
            scores = sc_ps.tile([4 * NSLOT, 512], dt.float32,
                                name="scores")
            bankA = bankA_pool.tile([128, 512], dt.float32, name="bankA")
            bankAgg = bankAgg_pool.tile([128, 512], dt.float32, name="bankAgg")
            bankZ = bankZ_pool.tile([128, 512], dt.float32, name="bankZ")
            bankD = bankD_pool.tile([128, 512], dt.float32, name="bankD")
            # bankA layout: v4 [0:512] (one 128-col region per block)
            nc.vector.memset(scores[:, :], 0.0)
            nc.vector.memset(bankZ[0:8, 0:512], 1.0)

            # stream chunk management
            chunk_tiles = {}

            def get_chunk(which, pool, ci):
                key = (which, ci)
                if key in chunk_tiles:
                    return chunk_tiles[key]
                c0 = ci * CHUNK * 128
                w = min(CHUNK * 128, NBF * 128 - c0)
                tl = pool.tile([128, CHUNK * 128], dt.bfloat16, name=which,
                               tag=which)
                src = {"hs": d_hs, "at": d_at, "Aa": d_Aa}[which]
                nc.sync.dma_start(out=tl[:, :w], in_=src[:, c0 : c0 + w])
                chunk_tiles[key] = tl
                return tl

            def chunk_slice(which, pool, fs, n):
                ci, off = divmod(fs, CHUNK)
                tl = get_chunk(which, pool, ci)
                return tl[:, off * 128 : (off + n) * 128]

            # per-tile state
            q_tiles = {}       # (tt, tile) -> Q sbuf tile
            tile_state = {}    # (tt, tile) -> dict(bankC, rels list)

            def emit_q(tt, t):
                key = (tt, t)
                if key in q_tiles:
                    return q_tiles[key]
                hdt = hdt_pool.tile([128, 128], dt.bfloat16, name="hdt",
                                    tag="hdt")
                nc.sync.dma_start(out=hdt[:], in_=d_hdT[tt][t, :, :])
                nc.tensor.matmul(bankD[:, 0:128], lhsT=hdt[:], rhs=s_wq[tt][:],
                                 start=True, stop=True)
                Q = q_pool.tile([128, 128], dt.bfloat16, name="Q", tag="Q")
                nc.scalar.copy(out=Q[:], in_=bankD[:, 0:128])
                q_tiles[key] = Q
                return Q

            tile_seq = [0]

            def get_tile_state(tt, t):
                key = (tt, t)
                if key not in tile_state:
                    tile_state[key] = {"rels": [], "half": 256 * (tile_seq[0] & 1)}
                    tile_seq[0] += 1
                return tile_state[key]

            def finalize_tile(tt, t):
                st = tile_state[(tt, t)]
                rels = st["rels"]
                orow = t * 128 if tt == 0 else (PT + t) * 128
                hrow = t_pool.tile([128, 128], dt.float32, name="hrow",
                                   tag="hrow")
                nc.sync.dma_start(out=hrow[:], in_=d_hrow[tt][t, :, :])
                out_s = t_pool.tile([128, 128], dt.float32, name="out_s",
                                    tag="out_s")
                if rels:
                    nr = len(rels)
                    hf = st["half"]
                    riof = [0, 1, 0]  # bank region per relation
                    c0 = hf + 128 * riof[rels[0]]
                    # zT rows 0-3 + the persistent ones row 4 -> SBUF
                    zT_sb = t_pool.tile([5, 256], dt.float32, name="zT_sb",
                                        tag="zT_sb")
                    nc.scalar.copy(out=zT_sb[:, 0 : 128 * nr],
                                   in_=bankZ[0:5, c0 : c0 + 128 * nr])
                    # one merged expand over all rels, one reciprocal
                    nc.tensor.matmul(
                        bankD[:, 256 : 256 + 128 * nr], lhsT=s_hsel4[:],
                        rhs=zT_sb[0:5, 0 : 128 * nr], start=True, stop=True)
                    rz_sb = t_pool.tile([128, 256], dt.float32,
                                        name="rz_sb", tag="rz_sb")
                    nc.vector.reciprocal_approx_fast(
                        out=rz_sb[:, 0 : 128 * nr],
                        in_=bankD[:, 256 : 256 + 128 * nr])
                    T_sbs = []
                    for pi, rel in enumerate(rels):
                        ri = riof[rel]
                        T_sb = t_pool.tile([128, 128], dt.bfloat16, name="T_sb",
                                           tag="T_sb")
                        nc.vector.tensor_tensor(
                            out=T_sb[:],
                            in0=bankAgg[:, hf + 128 * ri : hf + 128 * ri + 128],
                            in1=rz_sb[:, 128 * pi : 128 * pi + 128],
                            op=mybir.AluOpType.mult)
                        T_sbs.append(T_sb)
                    # out-MM accumulation pair kept adjacent: no other
                    # start=True matmul may land in bankD between them
                    for pi, T_sb in enumerate(T_sbs):
                        nc.tensor.matmul(bankD[:, 128:256], lhsT=T_sb[:],
                                         rhs=s_waT[tt][:],
                                         start=(pi == 0), stop=(pi == nr - 1))
                    nc.vector.scalar_tensor_tensor(
                        out=out_s[:], in0=hrow[:],
                        scalar=float(1.0 - alpha[tt]), in1=bankD[:, 128:256],
                        op0=mybir.AluOpType.mult, op1=mybir.AluOpType.add)
                else:
                    nc.vector.tensor_scalar(
                        out=out_s[:], in0=hrow[:],
                        scalar1=float(1.0 - alpha[tt]), scalar2=None,
                        op0=mybir.AluOpType.mult)
                nc.sync.dma_start(out=d_out[orow : orow + 128, :], in_=out_s[:])
                del tile_state[(tt, t)]

            # main superblock loop
            for sb in range(NSB):
                g0 = sb * NSLOT
                sb_groups = groups[g0 : g0 + NSLOT]
                ns = len(sb_groups)
                # ---- phase A ----
                for s, (fs, n, rel, tt, t, rfirst, rlast) in enumerate(sb_groups):
                    Q = emit_q(tt, t)
                    ec = n * 128
                    hs4 = chunk_slice("hs", hs_pool, fs, n)
                    at4 = chunk_slice("at", at_pool, fs, n)
                    kT = k_ps.tile([128, 512], dt.float32, name="kT", tag="kT")
                    nc.tensor.matmul(kT[:, :ec], lhsT=s_watt[rel][:], rhs=hs4,
                                     start=True, stop=True)
                    qxT = q_ps_pool.tile([128, 512], dt.float32, name="qxT",
                                         tag="qxT")
                    nc.tensor.matmul(qxT[:, :ec], lhsT=Q[:], rhs=at4,
                                     start=True, stop=True)
                    qxTs = qxs_pool.tile([128, 512], dt.float16,
                                         name="qxTs", tag="qxTs")
                    nc.scalar.copy(out=qxTs[:, :ec], in_=qxT[:, :ec])
                    prodT = prod_pool.tile([128, 512], dt.float16, name="prodT",
                                           tag="prodT")
                    nc.vector.tensor_tensor(out=prodT[:, :ec], in0=kT[:, :ec],
                                            in1=qxTs[:, :ec],
                                            op=mybir.AluOpType.mult)
                    nc.tensor.matmul(scores[:, :ec], lhsT=s_hmask(s)[:],
                                     rhs=prodT[:, :ec],
                                     start=(s == 0), stop=(s == ns - 1))
                # ---- exp (always full 64 rows: unused rows hold finite
                # stale scores; keeps escT NaN-free for the K=64 lhsT) ----
                escT = esc_pool.tile([4 * NSLOT, 512], dt.float16,
                                     name="escT", tag="escT")
                nc.scalar.activation(out=escT[:, :], in_=scores[:, :],
                                     func=mybir.ActivationFunctionType.Exp)
                # edge-major esc via xbar dma transpose: escET[e, 64j + k]
                # = escT[k, 128j + e]
                escET = escET_pool.tile([128, 4 * NSLOT * 4], dt.float16,
                                        name="escET", tag="escET")
                for j in range(4):
                    nc.sync.dma_start_transpose(
                        out=escET[:, 64 * j : 64 * j + 64],
                        in_=escT[:, 128 * j : 128 * j + 128])
                # ---- phase B ----
                for s, (fs, n, rel, tt, t, rfirst, rlast) in enumerate(sb_groups):
                    st = get_tile_state(tt, t)
                    ri = 0 if rel in (0, 2) else 1
                    if rel not in st["rels"]:
                        st["rels"].append(rel)
                    ec = n * 128
                    for j in range(n):
                        hsb = chunk_slice("hs", hs_pool, fs + j, 1)
                        nc.tensor.matmul(
                            bankA[:, 128 * j : 128 * j + 128], lhsT=hsb,
                            rhs=s_wmsg[rel][:], start=True, stop=True)
                    # esc for (slot s, block j) = escET[:, 64j+4s : +4]
                    escv = escET[:].rearrange("p (j r) -> p j r", r=64)[
                        :, 0:n, 4 * s : 4 * s + 4]
                    msg4 = msg_pool.tile([128, 512], dt.float16, name="msg4",
                                         tag="msg4")
                    nc.vector.tensor_tensor(
                        out=msg4[:, :ec].rearrange(
                            "p (j h r) -> p j h r", h=4, r=32),
                        in0=bankA[:, :ec].rearrange(
                            "p (j h r) -> p j h r", h=4, r=32),
                        in1=escv.to_broadcast([128, n, 4, 32]),
                        op=mybir.AluOpType.mult)
                    for j in range(n):
                        first = rfirst and j == 0
                        last = rlast and j == n - 1
                        Ab = chunk_slice("Aa", Aa_pool, fs + j, 1)
                        hf = st["half"]
                        nc.tensor.matmul(
                            bankAgg[:, hf + 128 * ri : hf + 128 * ri + 128],
                            lhsT=msg4[:, 128 * j : 128 * j + 128], rhs=Ab,
                            start=first, stop=last)
                        nc.tensor.matmul(
                            bankZ[0:4, hf + 128 * ri : hf + 128 * ri + 128],
                            lhsT=escET[:, 64 * j + 4 * s : 64 * j + 4 * s + 4],
                            rhs=Ab, start=first, stop=last)
                    if rlast:
                        # finalize when this was the tile's last relation run
                        is_tile_last = (rel == 2) or (tt == 0 and (
                            rel == 1 or (rel == 0 and nblk_w[t] == 0)))
                        if is_tile_last:
                            finalize_tile(tt, t)

            # tiles with no edges at all: pure skip-blend output
            seen = {(tt, t) for (_r, tt, t, _nb, _f, _ro) in runs}
            for tt, nt in ((0, PT), (1, AT)):
                for t in range(nt):
                    if (tt, t) not in seen:
                        get_tile_state(tt, t)
                        finalize_tile(tt, t)

    nc.compile()

    if os.environ.get("HGT_BUILD_ONLY"):
        return np.zeros((NPAP + NAUT, D), np.float32)

    in_maps = []
    for c in range(NCORES):
        in_maps.append({
            "hs_flat": hs_cores[c], "at_flat": at_cores[c],
            "Aa_flat": Aa_cores[c],
            "hdT_paper": hdT_p[c], "hdT_author": hdT_a[c],
            "hrow_paper": hrow_p[c], "hrow_author": hrow_a[c],
        })

    trace = bool(int(os.environ.get("HGT_TRACE", "0")))
    res = run_bass_kernel_spmd(nc, in_maps, list(range(NCORES)), trace=trace)
    LAST_RESULT["exec_time_ns"] = res.exec_time_ns
    LAST_RESULT["res"] = res
    LAST_RESULT["nc"] = nc
    LAST_RESULT["in_maps"] = in_maps

    out = np.empty((NPAP + NAUT, D), np.float32)
    for c in range(NCORES):
        o = np.asarray(res.results[c]["out"], np.float32)
        out[c * PPC : (c + 1) * PPC] = o[pos_p[c]]
        out[NPAP + c * APC : NPAP + (c + 1) * APC] = o[PT * 128 + pos_a[c]]
    return out
